# revision 1
# baseline (speedup 1.0000x reference)
"""Single-head causal attention on 8 TRN2 NeuronCores — v24 (out stores off the ACT queue).

Problem: x[B=8, T=2048, C=1024], Wq/Wk/Wv[C, H=64] (fp32)
  q = x@Wq; k = x@Wk; v = x@Wv
  wei = softmax(mask(q k^T * C^-0.5)); out = wei @ v       -> [B, T, H]

Sharding: data-parallel over batch, one batch element per core.

Per-core dataflow:
  - x loaded fp32 in 8x 1MB pieces on the gpsimd SWDGE queue.  SWDGE has
    its own completion-semaphore pool, so the loads never share lanes
    with the transposes (the HWDGE lane pool round-robins across queues
    and convoys the pipeline otherwise).  Cast fp32->bf16 per t-tile on
    DVE (2x mode) / ScalarE, xbar-transpose per t-tile on sync.
  - Projections packed so every S operand lands where it's needed with
    zero SBUF->SBUF copies:
      [Wk]    -> ka:  kT at partitions 0:64
      [Wq|Wv] -> qv:  qT at partitions 0:64, v at 64:128
    S^T block = ka_block.T @ qv[0:64]  (K=64, tile(0,0))
  - exp batched per block-pair [128,1024] across 2 PSUM banks.
  - causal mask = post-exp affine_select zeroing on bf16 pt (gpsimd).
  - PV accumulates [v|1]^T @ exp(S^T) -> row 64 gives sumexp for free;
    PE-transpose + reciprocal + scale for the final [T,H] output.
"""
import sys

sys.path.insert(0, "/opt/trn_rl_repo")

import numpy as np

import concourse.bass as bass
import concourse.mybir as mybir
import concourse.tile as tile
from concourse import bacc
from concourse.bass_utils import run_bass_kernel_spmd
from concourse.masks import make_identity

B, T, C, H = 8, 2048, 1024, 64
NTT = T // 128   # 16 t-tiles
NCT = C // 128   # 8  c-tiles
NCH = T // 512   # 4  t-chunks
SCALE = float(C) ** -0.5
VP = 80          # v_nat per-tile stride: 160B, 32B-aligned for xbar transpose

F32 = mybir.dt.float32
BF16 = mybir.dt.bfloat16


def build_nc(reps=1):
    nc = bacc.Bacc("TRN2", target_bir_lowering=False, debug=False,
                   dynamic_dma_scratch_size=49152)
    xD = nc.dram_tensor("x", [T, C], F32, kind="ExternalInput").ap()
    wqD = nc.dram_tensor("Wq", [128, NCT, H], F32, kind="ExternalInput").ap()
    wkD = nc.dram_tensor("Wk", [128, NCT, H], F32, kind="ExternalInput").ap()
    wvD = nc.dram_tensor("Wv", [128, NCT, H], F32, kind="ExternalInput").ap()
    outD = nc.dram_tensor("out", [T, H], F32, kind="ExternalOutput").ap()

    AF = mybir.ActivationFunctionType

    with tile.TileContext(nc) as tc:
        with (
            tc.tile_pool(name="const", bufs=1) as cpool,
            tc.tile_pool(name="xnat", bufs=1) as xnpool,
            tc.tile_pool(name="xt", bufs=1) as xtpool,
            tc.tile_pool(name="qk", bufs=1) as qkpool,
            tc.tile_pool(name="pt", bufs=4) as ptpool,
            tc.tile_pool(name="osb", bufs=3) as opool,
            tc.tile_pool(name="fin", bufs=2) as fpool,
        ):
            # ---- constants ----
            ident = cpool.tile([128, 128], F32)
            make_identity(nc, ident[:])
            # W loads ride the sync HWDGE ring (idle until the first big
            # transpose ~33us), so the two 4MB x loads lead the scalar
            # ring and start ~2us earlier.
            wqf = cpool.tile([128, NCT, H], F32)
            wvf = cpool.tile([128, NCT, H], F32)
            wkf = cpool.tile([128, NCT, H], F32)
            nc.sync.dma_start(wqf[:], wqD)
            nc.sync.dma_start(wvf[:], wvD)
            nc.sync.dma_start(wkf[:], wkD)
            wqv = cpool.tile([128, NCT, 128], BF16)
            wk = cpool.tile([128, NCT, H], BF16)
            nc.vector.tensor_copy(wqv[:, :, 0:H], wqf[:])
            nc.vector.tensor_copy(wqv[:, :, H:128], wvf[:])
            nc.vector.tensor_copy(wk[:], wkf[:])

            scrap = cpool.tile([128, 1], F32)

            for rep in range(reps):
                emit_body(nc, tc, xD, outD,
                          (wqv, wk, ident, scrap),
                          (xnpool, xtpool, qkpool, ptpool, opool, fpool))

    nc.compile()
    return nc


def emit_body(nc, tc, xD, outD, consts, pools):
    AF = mybir.ActivationFunctionType
    ALU = mybir.AluOpType
    wqv, wk, ident, scrap = consts
    xnpool, xtpool, qkpool, ptpool, opool, fpool = pools

    x_nat = xnpool.tile([128, NTT, C], F32, tag="xnat")
    x_natb = xnpool.tile([128, NTT, C], BF16, tag="xnatb")
    xt = xtpool.tile([128, NTT, NCT, 128], BF16, tag="xt")
    xR = xD.rearrange("(g p) c -> p g c", p=128)

    qv = qkpool.tile([128, T], BF16, tag="qv")    # rows 0:64 qT, 64:128 v
    ka = qkpool.tile([64, T], BF16, tag="ka")     # kT at partitions 0:64
    v_nat = qkpool.tile([128, NTT, VP], BF16, tag="vnat")
    nc.gpsimd.memset(v_nat[:, :, H:H + 1], 1.0)
    o_out = fpool.tile([128, NTT, H], F32, tag="oout")
    outR = outD.rearrange("(g p) h -> p g h", p=128)

    PIECES = ((0, 8), (8, 16))   # t-tile ranges per load piece

    def casth(h):
        # cast + transpose HALF of x (8 t-tiles) in one op each: t-tiles
        # share the same 128 partitions, so one xbar transpose of
        # [128, 8192] lands each 128-col group in its own (tile, c_grp)
        # slot of xt.  The scheduler serializes dynamic DMA op k+2 behind
        # transpose k regardless of queue, so with 2 loads + 2 transposes
        # the window never binds at all.
        nc.vector.tensor_copy(x_natb[:, 8 * h:8 * h + 8, :],
                              x_nat[:, 8 * h:8 * h + 8, :])
        nc.sync.dma_start(
            xt[:, 8 * h:8 * h + 8, :, :], x_natb[:, 8 * h:8 * h + 8, :],
            transpose=True,
        )

    with (
        tc.tile_pool(name="qkps", bufs=1, space="PSUM") as qkps,
        tc.tile_pool(name="aux", bufs=1, space="PSUM") as aux,
        tc.tile_pool(name="ops", bufs=2, space="PSUM") as ops,
        tc.tile_pool(name="stps", bufs=2, space="PSUM") as stps,
    ):
        vps = fps = aux

        def emit_warm(n):
            # PE warm-up gated on the weight cast (ready ~3us); keeps HAM
            # warm through the load lead-in until QKV(0).
            warm = qkps.tile([128, 512], F32, tag="psqk")
            for _ in range(n):
                nc.tensor.matmul(
                    warm[:], wqv[:, 0, :], wqv[:, 0:4, :].opt(),
                    start=True, stop=True,
                )

        def emit_qkv(ci):
            sl = slice(ci * 512, (ci + 1) * 512)
            ps_qv = vps.tile([128, 512], F32, tag="aux")
            for k in range(NCT):
                nc.tensor.matmul(
                    ps_qv[:], wqv[:, k, :], xt[:, ci * 4:(ci + 1) * 4, k, :],
                    start=(k == 0), stop=(k == NCT - 1),
                )
            nc.vector.tensor_copy(qv[:, sl], ps_qv[:])
            nc.sync.dma_start(
                v_nat[:, ci * 4:(ci + 1) * 4, 0:H], qv[64:128, sl],
                transpose=True,
            )
            ps_k_t = qkps.tile([128, 512], F32, tag="psqk")
            ps_k = ps_k_t[0:64, :]
            for k in range(NCT):
                nc.tensor.matmul(
                    ps_k[:], wk[:, k, :], xt[:, ci * 4:(ci + 1) * 4, k, :],
                    start=(k == 0), stop=(k == NCT - 1),
                )
            nc.vector.tensor_copy(ka[:, sl], ps_k[:])

        out_pcs = {}

        def emit_attn_multi(cis):
            # interleave the pair streams of several chunks so no chunk's
            # S matmuls sit behind another chunk's exp-gated PV matmuls
            # in the PE FIFO.
            order = []
            idx = 0
            while any(idx < 2 * ci + 2 for ci in cis):
                for ci in cis:
                    if idx < 2 * ci + 2:
                        order.append((ci, idx))
                idx += 1
            pending = []
            for ci, p in order:
                if ci not in out_pcs:
                    out_pc = ops.tile([H + 1, 512], F32, tag="outc")
                    out_pcs[ci] = out_pc
                out_pc = out_pcs[ci]
                nsb = 4 * ci + 4
                cl, cr = ci * 512, (ci + 1) * 512
                sbe, sbo = 2 * p, 2 * p + 1
                re, ro = sbe - 4 * ci, sbo - 4 * ci
                t0e, t0o = max(re, 0) * 128, max(ro, 0) * 128
                st = stps.tile([128, 1024], F32, tag="st")
                nc.tensor.matmul(
                    st[:, t0e:512],
                    ka[0:64, sbe * 128:(sbe + 1) * 128],
                    qv[0:64, cl + t0e:cr],
                    start=True, stop=True,
                )
                nc.tensor.matmul(
                    st[:, 512 + t0o:1024],
                    ka[0:64, sbo * 128:(sbo + 1) * 128],
                    qv[0:64, cl + t0o:cr],
                    start=True, stop=True,
                )
                pt = ptpool.tile([128, 1024], BF16, tag="pt")
                if re < 0:
                    nc.scalar.activation(
                        pt[:, 0:1024], st[:, 0:1024], AF.Exp, scale=SCALE)
                else:
                    nc.scalar.activation(
                        pt[:, t0e:512], st[:, t0e:512], AF.Exp, scale=SCALE)
                    nc.scalar.activation(
                        pt[:, 512 + t0o:1024], st[:, 512 + t0o:1024],
                        AF.Exp, scale=SCALE)
                    nc.gpsimd.affine_select(
                        out=pt[:, t0e:t0e + 128], in_=pt[:, t0e:t0e + 128],
                        compare_op=ALU.is_ge, fill=0.0,
                        base=0, pattern=[[1, 128]], channel_multiplier=-1,
                    )
                    nc.gpsimd.affine_select(
                        out=pt[:, 512 + t0o:512 + t0o + 128],
                        in_=pt[:, 512 + t0o:512 + t0o + 128],
                        compare_op=ALU.is_ge, fill=0.0,
                        base=0, pattern=[[1, 128]], channel_multiplier=-1,
                    )
                if pending:
                    for args, kw in pending:
                        nc.tensor.matmul(*args, **kw)
                pending = [
                    ((out_pc[:, t0e:512], v_nat[:, sbe, 0:H + 1],
                      pt[:, t0e:512]),
                     dict(start=(sbe == 0), stop=False)),
                    ((out_pc[:, t0o:512], v_nat[:, sbo, 0:H + 1],
                      pt[:, 512 + t0o:1024]),
                     dict(start=False, stop=(sbo == nsb - 1))),
                ]
            for args, kw in pending:
                nc.tensor.matmul(*args, **kw)

        def emit_attn_core(ci):
            out_pc = ops.tile([H + 1, 512], F32, tag="outc")
            out_pcs[ci] = out_pc
            npair = 2 * ci + 2
            nsb = 4 * ci + 4
            cl, cr = ci * 512, (ci + 1) * 512
            pending = []
            for p in range(npair):
                sbe, sbo = 2 * p, 2 * p + 1
                re, ro = sbe - 4 * ci, sbo - 4 * ci
                t0e, t0o = max(re, 0) * 128, max(ro, 0) * 128
                st = stps.tile([128, 1024], F32, tag="st")
                nc.tensor.matmul(
                    st[:, t0e:512],
                    ka[0:64, sbe * 128:(sbe + 1) * 128],
                    qv[0:64, cl + t0e:cr],
                    start=True, stop=True,
                )
                nc.tensor.matmul(
                    st[:, 512 + t0o:1024],
                    ka[0:64, sbo * 128:(sbo + 1) * 128],
                    qv[0:64, cl + t0o:cr],
                    start=True, stop=True,
                )
                pt = ptpool.tile([128, 1024], BF16, tag="pt")
                if re < 0:  # fully off-diagonal pair: one batched exp
                    nc.scalar.activation(
                        pt[:, 0:1024], st[:, 0:1024], AF.Exp, scale=SCALE)
                else:
                    nc.scalar.activation(
                        pt[:, t0e:512], st[:, t0e:512], AF.Exp, scale=SCALE)
                    nc.scalar.activation(
                        pt[:, 512 + t0o:1024], st[:, 512 + t0o:1024],
                        AF.Exp, scale=SCALE)
                    # zero upper triangle of the diagonal 128-blocks
                    nc.gpsimd.affine_select(
                        out=pt[:, t0e:t0e + 128], in_=pt[:, t0e:t0e + 128],
                        compare_op=ALU.is_ge, fill=0.0,
                        base=0, pattern=[[1, 128]], channel_multiplier=-1,
                    )
                    nc.gpsimd.affine_select(
                        out=pt[:, 512 + t0o:512 + t0o + 128],
                        in_=pt[:, 512 + t0o:512 + t0o + 128],
                        compare_op=ALU.is_ge, fill=0.0,
                        base=0, pattern=[[1, 128]], channel_multiplier=-1,
                    )
                if pending:
                    for args, kw in pending:
                        nc.tensor.matmul(*args, **kw)
                pending = [
                    ((out_pc[:, t0e:512], v_nat[:, sbe, 0:H + 1],
                      pt[:, t0e:512]),
                     dict(start=(sbe == 0), stop=False)),
                    ((out_pc[:, t0o:512], v_nat[:, sbo, 0:H + 1],
                      pt[:, 512 + t0o:1024]),
                     dict(start=False, stop=(sbo == nsb - 1))),
                ]
            for args, kw in pending:
                nc.tensor.matmul(*args, **kw)

        def emit_attn_out(ci):
            out_pc = out_pcs[ci]
            o_c = opool.tile([H + 1, 512], F32, tag="osb")
            nc.vector.tensor_copy(o_c[:], out_pc[:])
            fin_t = fps.tile([128, 4, 128], F32, tag="aux")
            fin4 = fin_t[:, :, 0:H + 1]
            for rr in range(4):
                nc.tensor.transpose(
                    fin4[:, rr, :],
                    o_c[:, rr * 128:(rr + 1) * 128],
                    ident[0:H + 1, 0:H + 1],
                )
            rcp = fpool.tile([128, 4, 1], F32, tag="rcp")
            nc.vector.reciprocal(rcp[:], fin4[:, :, H:H + 1])
            nc.vector.tensor_tensor(
                o_out[:, ci * 4:(ci + 1) * 4, :], fin4[:, :, 0:H],
                rcp[:].to_broadcast([128, 4, H]), op=ALU.mult,
            )
            # store on the sync ring: a store's wait in the strict-FIFO
            # ACT queue would block every tail exp queued behind it.
            nc.sync.dma_start(
                outR[:, ci * 4:(ci + 1) * 4, :],
                o_out[:, ci * 4:(ci + 1) * 4, :],
            )

        # ---- three x loads queued up-front on the scalar HWDGE ring
        # (HWDGE streams ~360 GB/s; SWDGE measured only ~175 GB/s here).
        for lo, hi in PIECES:
            nc.scalar.dma_start(x_nat[:, lo:hi, :], xR[:, lo:hi, :])
        # table preload: first Exp triggers ACT_TABLE_LOAD early (after
        # the load issues so it doesn't head-block them in the ACT FIFO)
        nc.scalar.activation(scrap[:], ident[:, 0:1], AF.Exp)

        emit_warm(34)
        casth(0)
        casth(1)
        emit_qkv(0)
        emit_attn_core(0)
        emit_qkv(1)
        emit_attn_core(1)
        emit_attn_out(0)
        emit_attn_out(1)
        emit_qkv(2)
        emit_qkv(3)
        emit_attn_multi((2, 3))
        emit_attn_out(2)
        emit_attn_out(3)


_NC = None


def kernel(x, Wq, Wk, Wv):
    global _NC
    if _NC is None:
        _NC = build_nc()
    def wperm(W):
        return np.ascontiguousarray(
            np.asarray(W, dtype=np.float32).reshape(NCT, 128, H)
            .transpose(1, 0, 2))

    WqP, WkP, WvP = wperm(Wq), wperm(Wk), wperm(Wv)
    in_maps = [
        {
            "x": np.ascontiguousarray(x[b], dtype=np.float32),
            "Wq": WqP, "Wk": WkP, "Wv": WvP,
        }
        for b in range(B)
    ]
    res = run_bass_kernel_spmd(_NC, in_maps, core_ids=list(range(B)))
    return np.stack([res.results[b]["out"] for b in range(B)], axis=0)



# revision 5
# speedup vs baseline: 1.3645x; 1.3645x over previous
"""Single-head causal attention on 8 TRN2 NeuronCores — v25.

Problem: x[B=8, T=2048, C=1024], Wq/Wk/Wv[C, H=64] (fp32)
  q = x@Wq; k = x@Wk; v = x@Wv
  wei = softmax(mask(q k^T * C^-0.5)); out = wei @ v       -> [B, T, H]

Sharding: data-parallel over batch, one batch element per core.

v25 redesign vs v24:
  - x is marshaled HOST-side: uploaded pre-transposed (x^T) so the
    device needs no fp32 load, no DVE cast, and no xbar DMA transposes
    (which serialize against all other DMA traffic on mode switches).
    Two copies go up: bf16 x^T [128c, NCT, T] (4MB) for the v
    projection + S operands, and an fp8e4 even/odd-c byte-packed
    x^T [128, KT, T, 2] (2MB) for the q/k projections.
  - q/k projections run as fp8 DoubleRow matmuls (0.5 cyc/row): the
    byte-packed layout puts c=2p+i at (partition p, byte i), matching
    DoubleRow's [K, 2, N] two-subtile contraction exactly.
  - PV: fully-below-diagonal pairs use fp8 DoubleRow (exp -> pt8 fp8
    directly on ACT; v8 cast of v), one matmul per pair; diagonal
    pairs keep the bf16 path with affine_select masking.  Rows of
    chunk 0 stay all-bf16 (out[0]=v[0] exactly -> fp8 v would put ~6%
    error there; for t>=512 the softmax averaging buries it).
  - v_nat comes from PE transposes (bf16 identity), not the DMA xbar.
  - Weights are host-packed (fp8 DoubleRow layout / bf16) and loaded
    via gpsimd SWDGE so the scalar HWDGE ring is x-only.
Measured rel err (numpy emulation): 6.3e-3 vs 2e-2 gate.
"""
import sys

sys.path.insert(0, "/opt/trn_rl_repo")

import numpy as np
import ml_dtypes

import concourse.bass as bass
import concourse.mybir as mybir
import concourse.tile as tile
from concourse import bacc
from concourse.bass_utils import run_bass_kernel_spmd
from concourse.masks import make_identity

B, T, C, H = 8, 2048, 1024, 64
NTT = T // 128   # 16 t-tiles
NCT = C // 128   # 8  c-tiles (bf16 path)
KT = C // 256    # 4  doublerow c-tiles (fp8 path)
NCH = T // 512   # 4  t-chunks
SCALE = float(C) ** -0.5

F32 = mybir.dt.float32
BF16 = mybir.dt.bfloat16
FP8 = mybir.dt.float8e4
DR = mybir.MatmulPerfMode.DoubleRow


def build_nc(reps=1):
    nc = bacc.Bacc("TRN2", target_bir_lowering=False, debug=False,
                   dynamic_dma_scratch_size=49152)
    x8D = nc.dram_tensor("x8", [128, KT, T, 2], FP8, kind="ExternalInput").ap()
    xtD = nc.dram_tensor("xt", [128, NCT, T], BF16, kind="ExternalInput").ap()
    wq8D = nc.dram_tensor("wq8", [128, KT, 2, H], FP8, kind="ExternalInput").ap()
    wk8D = nc.dram_tensor("wk8", [128, KT, 2, H], FP8, kind="ExternalInput").ap()
    wvD = nc.dram_tensor("wv", [128, NCT, H], BF16, kind="ExternalInput").ap()
    outD = nc.dram_tensor("out", [T, H], F32, kind="ExternalOutput").ap()

    AF = mybir.ActivationFunctionType

    with tile.TileContext(nc) as tc:
        with (
            tc.tile_pool(name="const", bufs=1) as cpool,
            tc.tile_pool(name="xin", bufs=1) as xpool,
            tc.tile_pool(name="qk", bufs=1) as qkpool,
            tc.tile_pool(name="pt", bufs=4) as ptpool,
            tc.tile_pool(name="osb", bufs=3) as opool,
            tc.tile_pool(name="fin", bufs=2) as fpool,
        ):
            # x loads lead the scalar HWDGE ring: per-chunk pieces so the
            # first QKV can start ~4us into the load stream.
            x8 = xpool.tile([128, KT, T, 2], FP8, tag="x8")
            xt = xpool.tile([128, NCT, T], BF16, tag="xt")
            for ci in range(NCH):
                sl = slice(ci * 512, (ci + 1) * 512)
                nc.scalar.dma_start(x8[:, :, sl, :], x8D[:, :, sl, :])
                nc.scalar.dma_start(xt[:, :, sl], xtD[:, :, sl])

            # weights ride the gpsimd SWDGE (own engine + sem pool; tiny)
            wq8 = cpool.tile([128, KT, 2, H], FP8)
            wk8 = cpool.tile([128, KT, 2, H], FP8)
            wv = cpool.tile([128, NCT, H], BF16)
            nc.gpsimd.dma_start(wq8[:], wq8D)
            nc.gpsimd.dma_start(wk8[:], wk8D)
            nc.gpsimd.dma_start(wv[:], wvD)

            ident = cpool.tile([128, 128], F32)
            make_identity(nc, ident[:])
            identb = cpool.tile([128, 64], BF16)
            nc.vector.tensor_copy(identb[64:128, :], ident[64:128, 64:128])

            scrap = cpool.tile([128, 1], F32)
            # first Exp triggers ACT_TABLE_LOAD early (after load issue so
            # it doesn't head-block the x loads in any DMA path)
            nc.scalar.activation(scrap[:], ident[:, 0:1], AF.Exp)

            for rep in range(reps):
                emit_body(nc, tc, outD,
                          (x8, xt, wq8, wk8, wv, ident, identb),
                          (qkpool, ptpool, opool, fpool))

    nc.compile()
    return nc


def emit_body(nc, tc, outD, consts, pools):
    AF = mybir.ActivationFunctionType
    ALU = mybir.AluOpType
    x8, xt, wq8, wk8, wv, ident, identb = consts
    qkpool, ptpool, opool, fpool = pools

    qa = qkpool.tile([64, T], BF16, tag="qa")
    ka = qkpool.tile([64, T], BF16, tag="ka")
    va = qkpool.tile([128, T], BF16, tag="va")    # rows 64:128 hold v^T
    v_nat = qkpool.tile([128, NTT, H + 1], BF16, tag="vnat")
    VP8 = 80   # fp8 v stride: dual-fp8 LDWEIGHTS needs even, 16-aligned subtile step
    v8_nat = qkpool.tile([128, NTT, VP8], FP8, tag="v8nat")
    nc.gpsimd.memset(v_nat[:, :, H:H + 1], 1.0)
    nc.gpsimd.memset(v8_nat[:, :, H:H + 1], 1.0)
    o_out = fpool.tile([128, NTT, H], F32, tag="oout")
    outR = outD.rearrange("(g p) h -> p g h", p=128)

    with (
        tc.tile_pool(name="qkvps", bufs=1, space="PSUM") as qkvps,
        tc.tile_pool(name="vtps", bufs=1, space="PSUM") as vtps,
        tc.tile_pool(name="stps", bufs=2, space="PSUM") as stps,
        tc.tile_pool(name="ops", bufs=1, space="PSUM") as ops,
    ):
        def emit_warm(n):
            # fp32 identity matmuls keep the PE p-state ramping from ~7us
            # until the first QKV; gated only on make_identity.
            warm = stps.tile([128, 1024], F32, tag="st")
            for _ in range(n):
                nc.tensor.matmul(
                    warm[0:128, 0:128], ident[:], ident[:],
                    start=True, stop=True,
                )

        def emit_qkv(ci):
            sl = slice(ci * 512, (ci + 1) * 512)
            # v first (bf16): va feeds the PE transposes below
            ps_v = qkvps.tile([64, 512], F32, tag="psv")
            for c in range(NCT):
                nc.tensor.matmul(
                    ps_v[:], wv[:, c, :], xt[:, c, sl],
                    start=(c == 0), stop=(c == NCT - 1),
                )
            nc.vector.tensor_copy(va[64:128, sl], ps_v[:])
            # q, k: fp8 DoubleRow over 4 256-wide c-tiles
            ps_q = qkvps.tile([64, 512], F32, tag="psq")
            for kt in range(KT):
                nc.tensor.matmul(
                    ps_q[:], wq8[:, kt, :, :],
                    x8[:, kt, sl, :].rearrange("p t i -> p i t"),
                    start=(kt == 0), stop=(kt == KT - 1), perf_mode=DR,
                )
            nc.vector.tensor_copy(qa[:, sl], ps_q[:])
            ps_k = qkvps.tile([64, 512], F32, tag="psv")
            for kt in range(KT):
                nc.tensor.matmul(
                    ps_k[:], wk8[:, kt, :, :],
                    x8[:, kt, sl, :].rearrange("p t i -> p i t"),
                    start=(kt == 0), stop=(kt == KT - 1), perf_mode=DR,
                )
            nc.vector.tensor_copy(ka[:, sl], ps_k[:])
            # v^T -> v natural via PE transposes (no DMA xbar!)
            ps_vt = vtps.tile([128, 4, H], BF16, tag="vt")
            for r in range(4):
                nc.tensor.transpose(
                    ps_vt[:, r, :],
                    va[64:128, ci * 512 + r * 128: ci * 512 + (r + 1) * 128],
                    identb[64:128, :],
                )
            nc.vector.tensor_copy(v_nat[:, 4 * ci:4 * ci + 4, 0:H], ps_vt[:])
            nc.vector.tensor_copy(v8_nat[:, 4 * ci:4 * ci + 4, 0:H], ps_vt[:])

        out_tiles = {}

        def emit_attn_pair(ci, p, pending):
            """Emit S+exp for pair p of chunk ci; flush `pending` PV matmuls
            after exp is queued; return this pair's PV matmuls as new pending."""
            if ci not in out_tiles:
                out_tiles[ci] = ops.tile([128, 512], F32, tag="outc",
                                         name=f"outc{ci}")
            out_pc = out_tiles[ci][0:H + 1, :]
            nsb = 4 * ci + 4
            cl, cr = ci * 512, (ci + 1) * 512
            sbe, sbo = 2 * p, 2 * p + 1
            re, ro = sbe - 4 * ci, sbo - 4 * ci
            st = stps.tile([128, 1024], F32, tag="st")
            if re < 0:
                # fully below the diagonal: full-width S, fp8 exp, 1 DR PV
                nc.tensor.matmul(
                    st[:, 0:512],
                    ka[0:64, sbe * 128:(sbe + 1) * 128], qa[0:64, cl:cr],
                    start=True, stop=True,
                )
                nc.tensor.matmul(
                    st[:, 512:1024],
                    ka[0:64, sbo * 128:(sbo + 1) * 128], qa[0:64, cl:cr],
                    start=True, stop=True,
                )
                pt8 = ptpool.tile([128, 1024], FP8, tag="pt8")
                nc.scalar.activation(pt8[:], st[:], AF.Exp, scale=SCALE)
                for args, kw in pending:
                    nc.tensor.matmul(*args, **kw)
                return [
                    ((out_pc[:, :], v8_nat[:, sbe:sbe + 2, 0:H + 1],
                      pt8[:].rearrange("q (i t) -> q i t", i=2)),
                     dict(start=(sbe == 0), stop=False, perf_mode=DR,
                          skip_group_check=True)),
                ]
            # diagonal pair: trimmed S, bf16 exp + affine_select mask
            t0e, t0o = max(re, 0) * 128, max(ro, 0) * 128
            nc.tensor.matmul(
                st[:, t0e:512],
                ka[0:64, sbe * 128:(sbe + 1) * 128], qa[0:64, cl + t0e:cr],
                start=True, stop=True,
            )
            nc.tensor.matmul(
                st[:, 512 + t0o:1024],
                ka[0:64, sbo * 128:(sbo + 1) * 128], qa[0:64, cl + t0o:cr],
                start=True, stop=True,
            )
            pt = ptpool.tile([128, 1024], BF16, tag="pt")
            nc.scalar.activation(
                pt[:, t0e:512], st[:, t0e:512], AF.Exp, scale=SCALE)
            nc.scalar.activation(
                pt[:, 512 + t0o:1024], st[:, 512 + t0o:1024],
                AF.Exp, scale=SCALE)
            nc.gpsimd.affine_select(
                out=pt[:, t0e:t0e + 128], in_=pt[:, t0e:t0e + 128],
                compare_op=ALU.is_ge, fill=0.0,
                base=0, pattern=[[1, 128]], channel_multiplier=-1,
            )
            nc.gpsimd.affine_select(
                out=pt[:, 512 + t0o:512 + t0o + 128],
                in_=pt[:, 512 + t0o:512 + t0o + 128],
                compare_op=ALU.is_ge, fill=0.0,
                base=0, pattern=[[1, 128]], channel_multiplier=-1,
            )
            for args, kw in pending:
                nc.tensor.matmul(*args, **kw)
            return [
                ((out_pc[:, t0e:512], v_nat[:, sbe, 0:H + 1],
                  pt[:, t0e:512]),
                 dict(start=(sbe == 0), stop=False, skip_group_check=True)),
                ((out_pc[:, t0o:512], v_nat[:, sbo, 0:H + 1],
                  pt[:, 512 + t0o:1024]),
                 dict(start=False, stop=(sbo == nsb - 1),
                      skip_group_check=True)),
            ]

        def emit_attn_core(ci):
            pending = []
            for p in range(2 * ci + 2):
                pending = emit_attn_pair(ci, p, pending)
            for args, kw in pending:
                nc.tensor.matmul(*args, **kw)

        def emit_attn_multi(cis):
            # interleave pair streams so no chunk's S matmuls queue behind
            # another chunk's exp-gated PV matmuls in the PE FIFO.
            order = []
            idx = 0
            while any(idx < 2 * ci + 2 for ci in cis):
                for ci in cis:
                    if idx < 2 * ci + 2:
                        order.append((ci, idx))
                idx += 1
            pending = []
            for ci, p in order:
                pending = emit_attn_pair(ci, p, pending)
            for args, kw in pending:
                nc.tensor.matmul(*args, **kw)

        def emit_attn_out(ci):
            out_tile = out_tiles[ci]
            out_pc = out_tile[0:H + 1, :]
            o_c = opool.tile([H + 1, 512], F32, tag="osb")
            nc.vector.tensor_copy(o_c[:], out_pc[:])
            # reuse the same PSUM bank for the transposed result
            fin4 = out_tile[:, 0:4 * (H + 1)].rearrange(
                "q (r h) -> q r h", h=H + 1)
            for rr in range(4):
                nc.tensor.transpose(
                    fin4[:, rr, :],
                    o_c[:, rr * 128:(rr + 1) * 128],
                    ident[0:H + 1, 0:H + 1],
                )
            rcp = fpool.tile([128, 4, 1], F32, tag="rcp")
            nc.vector.reciprocal(rcp[:], fin4[:, :, H:H + 1])
            nc.vector.tensor_tensor(
                o_out[:, ci * 4:(ci + 1) * 4, :], fin4[:, :, 0:H],
                rcp[:].to_broadcast([128, 4, H]), op=ALU.mult,
            )
            nc.sync.dma_start(
                outR[:, ci * 4:(ci + 1) * 4, :],
                o_out[:, ci * 4:(ci + 1) * 4, :],
            )
            del out_tiles[ci]

        emit_warm(14)
        emit_qkv(0)
        emit_attn_core(0)
        emit_qkv(1)
        emit_attn_core(1)
        emit_attn_out(0)
        emit_attn_out(1)
        emit_qkv(2)
        emit_qkv(3)
        emit_attn_multi((2, 3))
        emit_attn_out(2)
        emit_attn_out(3)


BF = ml_dtypes.bfloat16
F8 = ml_dtypes.float8_e4m3


def prep_weights(Wq, Wk, Wv):
    Wq = np.asarray(Wq, dtype=np.float32)
    Wk = np.asarray(Wk, dtype=np.float32)
    Wv = np.asarray(Wv, dtype=np.float32)

    def w8(W):
        # [128, KT, 2, H] with [p, kt, i, h] = W[256kt + 2p + i, h]
        return np.ascontiguousarray(
            W.astype(F8).reshape(KT, 128, 2, H).transpose(1, 0, 2, 3))

    wv = np.ascontiguousarray(
        Wv.astype(BF).reshape(NCT, 128, H).transpose(1, 0, 2))
    return {"wq8": w8(Wq), "wk8": w8(Wk), "wv": wv}


def prep_x(xb):
    xb = np.asarray(xb, dtype=np.float32)
    # xt [128, NCT, T] bf16 with [p, ct, t] = x[t, 128ct + p]
    xt = np.ascontiguousarray(
        xb.astype(BF).reshape(T, NCT, 128).transpose(2, 1, 0))
    # x8 [128, KT, T, 2] fp8 with [p, kt, t, i] = x[t, 256kt + 2p + i]
    x8 = np.ascontiguousarray(
        xb.astype(F8).reshape(T, KT, 128, 2).transpose(2, 1, 0, 3))
    return {"x8": x8, "xt": xt}


def make_in_maps(x, Wq, Wk, Wv):
    wmap = prep_weights(Wq, Wk, Wv)
    return [{**prep_x(x[b]), **wmap} for b in range(B)]


_NC = None


def kernel(x, Wq, Wk, Wv):
    global _NC
    if _NC is None:
        _NC = build_nc()
    in_maps = make_in_maps(x, Wq, Wk, Wv)
    res = run_bass_kernel_spmd(_NC, in_maps, core_ids=list(range(B)))
    return np.stack([res.results[b]["out"] for b in range(B)], axis=0)


# revision 6
# speedup vs baseline: 1.4770x; 1.0824x over previous
"""Single-head causal attention on 8 TRN2 NeuronCores — v25.

Problem: x[B=8, T=2048, C=1024], Wq/Wk/Wv[C, H=64] (fp32)
  q = x@Wq; k = x@Wk; v = x@Wv
  wei = softmax(mask(q k^T * C^-0.5)); out = wei @ v       -> [B, T, H]

Sharding: data-parallel over batch, one batch element per core.

v25 redesign vs v24:
  - x is marshaled HOST-side: uploaded pre-transposed (x^T) so the
    device needs no fp32 load, no DVE cast, and no xbar DMA transposes
    (which serialize against all other DMA traffic on mode switches).
    Two copies go up: bf16 x^T [128c, NCT, T] (4MB) for the v
    projection + S operands, and an fp8e4 even/odd-c byte-packed
    x^T [128, KT, T, 2] (2MB) for the q/k projections.
  - q/k projections run as fp8 DoubleRow matmuls (0.5 cyc/row): the
    byte-packed layout puts c=2p+i at (partition p, byte i), matching
    DoubleRow's [K, 2, N] two-subtile contraction exactly.
  - PV: fully-below-diagonal pairs use fp8 DoubleRow (exp -> pt8 fp8
    directly on ACT; v8 cast of v), one matmul per pair; diagonal
    pairs keep the bf16 path with affine_select masking.  Rows of
    chunk 0 stay all-bf16 (out[0]=v[0] exactly -> fp8 v would put ~6%
    error there; for t>=512 the softmax averaging buries it).
  - v_nat comes from PE transposes (bf16 identity), not the DMA xbar.
  - Weights are host-packed (fp8 DoubleRow layout / bf16) and loaded
    via gpsimd SWDGE so the scalar HWDGE ring is x-only.
Measured rel err (numpy emulation): 6.3e-3 vs 2e-2 gate.
"""
import sys

sys.path.insert(0, "/opt/trn_rl_repo")

import numpy as np
import ml_dtypes

import concourse.bass as bass
import concourse.mybir as mybir
import concourse.tile as tile
from concourse import bacc
from concourse.bass_utils import run_bass_kernel_spmd
from concourse.masks import make_identity

B, T, C, H = 8, 2048, 1024, 64
NTT = T // 128   # 16 t-tiles
NCT = C // 128   # 8  c-tiles (bf16 path)
KT = C // 256    # 4  doublerow c-tiles (fp8 path)
NCH = T // 512   # 4  t-chunks
SCALE = float(C) ** -0.5

F32 = mybir.dt.float32
BF16 = mybir.dt.bfloat16
FP8 = mybir.dt.float8e4
DR = mybir.MatmulPerfMode.DoubleRow


def build_nc(reps=1):
    nc = bacc.Bacc("TRN2", target_bir_lowering=False, debug=False,
                   dynamic_dma_scratch_size=49152)
    x8D = nc.dram_tensor("x8", [128, NCH, KT, 512, 2], FP8,
                         kind="ExternalInput").ap()
    xtD = nc.dram_tensor("xt", [128, NCH, NCT, 512], BF16,
                         kind="ExternalInput").ap()
    wq8D = nc.dram_tensor("wq8", [128, KT, 2, H], FP8, kind="ExternalInput").ap()
    wk8D = nc.dram_tensor("wk8", [128, KT, 2, H], FP8, kind="ExternalInput").ap()
    wvD = nc.dram_tensor("wv", [128, NCT, H], BF16, kind="ExternalInput").ap()
    outD = nc.dram_tensor("out", [T, H], F32, kind="ExternalOutput").ap()

    AF = mybir.ActivationFunctionType

    with tile.TileContext(nc) as tc:
        with (
            tc.tile_pool(name="const", bufs=1) as cpool,
            tc.tile_pool(name="xin", bufs=1) as xpool,
            tc.tile_pool(name="qk", bufs=1) as qkpool,
            tc.tile_pool(name="pt", bufs=4) as ptpool,
            tc.tile_pool(name="osb", bufs=3) as opool,
            tc.tile_pool(name="fin", bufs=2) as fpool,
        ):
            # x loads lead the scalar HWDGE ring: per-chunk pieces so the
            # first QKV can start ~4us into the load stream.
            # chunk-contiguous per partition (4KB / 8KB bursts), on the
            # sync HWDGE ring: descriptor generation runs on the SP engine,
            # keeping the Activation engine free for exp from the start.
            x8 = xpool.tile([128, NCH, KT, 512, 2], FP8, tag="x8")
            xt = xpool.tile([128, NCH, NCT, 512], BF16, tag="xt")
            for ci in range(NCH):
                nc.sync.dma_start(x8[:, ci], x8D[:, ci])
                nc.sync.dma_start(xt[:, ci], xtD[:, ci])

            # weights ride the gpsimd SWDGE (own engine + sem pool; tiny)
            wq8 = cpool.tile([128, KT, 2, H], FP8)
            wk8 = cpool.tile([128, KT, 2, H], FP8)
            wv = cpool.tile([128, NCT, H], BF16)
            nc.gpsimd.dma_start(wq8[:], wq8D)
            nc.gpsimd.dma_start(wk8[:], wk8D)
            nc.gpsimd.dma_start(wv[:], wvD)

            ident = cpool.tile([128, 128], F32)
            make_identity(nc, ident[:])
            identb = cpool.tile([128, 64], BF16)
            nc.vector.tensor_copy(identb[64:128, :], ident[64:128, 64:128])

            scrap = cpool.tile([128, 1], F32)
            # first Exp triggers ACT_TABLE_LOAD early (after load issue so
            # it doesn't head-block the x loads in any DMA path)
            nc.scalar.activation(scrap[:], ident[:, 0:1], AF.Exp)

            for rep in range(reps):
                emit_body(nc, tc, outD,
                          (x8, xt, wq8, wk8, wv, ident, identb),
                          (qkpool, ptpool, opool, fpool))

    nc.compile()
    return nc


def emit_body(nc, tc, outD, consts, pools):
    AF = mybir.ActivationFunctionType
    ALU = mybir.AluOpType
    x8, xt, wq8, wk8, wv, ident, identb = consts
    qkpool, ptpool, opool, fpool = pools

    qa = qkpool.tile([64, T], BF16, tag="qa")
    ka = qkpool.tile([64, T], BF16, tag="ka")
    va = qkpool.tile([128, T], BF16, tag="va")    # rows 64:128 hold v^T
    v_nat = qkpool.tile([128, NTT, H + 1], BF16, tag="vnat")
    VP8 = 80   # fp8 v stride: dual-fp8 LDWEIGHTS needs even, 16-aligned subtile step
    v8_nat = qkpool.tile([128, NTT, VP8], FP8, tag="v8nat")
    nc.gpsimd.memset(v_nat[:, :, H:H + 1], 1.0)
    nc.gpsimd.memset(v8_nat[:, :, H:H + 1], 1.0)
    o_out = fpool.tile([128, NTT, H], F32, tag="oout")
    outR = outD.rearrange("(g p) h -> p g h", p=128)

    with (
        tc.tile_pool(name="qkvps", bufs=1, space="PSUM") as qkvps,
        tc.tile_pool(name="vtps", bufs=1, space="PSUM") as vtps,
        tc.tile_pool(name="stps", bufs=2, space="PSUM") as stps,
        tc.tile_pool(name="ops", bufs=1, space="PSUM") as ops,
    ):
        def emit_warm(n):
            # fp32 identity matmuls keep the PE p-state ramping from ~7us
            # until the first QKV; gated only on make_identity.
            warm = stps.tile([128, 1024], F32, tag="st")
            for _ in range(n):
                nc.tensor.matmul(
                    warm[0:128, 0:128], ident[:], ident[:],
                    start=True, stop=True,
                )

        def emit_qkv(ci):
            sl = slice(ci * 512, (ci + 1) * 512)
            # v first (bf16): va feeds the PE transposes below
            ps_v = qkvps.tile([64, 512], F32, tag="psv")
            for c in range(NCT):
                nc.tensor.matmul(
                    ps_v[:], wv[:, c, :], xt[:, ci, c, :],
                    start=(c == 0), stop=(c == NCT - 1),
                )
            nc.vector.tensor_copy(va[64:128, sl], ps_v[:])
            # q, k: fp8 DoubleRow over 4 256-wide c-tiles
            ps_q = qkvps.tile([64, 512], F32, tag="psq")
            for kt in range(KT):
                nc.tensor.matmul(
                    ps_q[:], wq8[:, kt, :, :],
                    x8[:, ci, kt, :, :].rearrange("p t i -> p i t"),
                    start=(kt == 0), stop=(kt == KT - 1), perf_mode=DR,
                )
            nc.vector.tensor_copy(qa[:, sl], ps_q[:])
            ps_k = qkvps.tile([64, 512], F32, tag="psv")
            for kt in range(KT):
                nc.tensor.matmul(
                    ps_k[:], wk8[:, kt, :, :],
                    x8[:, ci, kt, :, :].rearrange("p t i -> p i t"),
                    start=(kt == 0), stop=(kt == KT - 1), perf_mode=DR,
                )
            nc.vector.tensor_copy(ka[:, sl], ps_k[:])
            # v^T -> v natural via PE transposes (no DMA xbar!)
            ps_vt = vtps.tile([128, 4, H], BF16, tag="vt")
            for r in range(4):
                nc.tensor.transpose(
                    ps_vt[:, r, :],
                    va[64:128, ci * 512 + r * 128: ci * 512 + (r + 1) * 128],
                    identb[64:128, :],
                )
            nc.vector.tensor_copy(v_nat[:, 4 * ci:4 * ci + 4, 0:H], ps_vt[:])
            nc.vector.tensor_copy(v8_nat[:, 4 * ci:4 * ci + 4, 0:H], ps_vt[:])

        out_tiles = {}

        def emit_attn_pair(ci, p, pending):
            """Emit S+exp for pair p of chunk ci; flush `pending` PV matmuls
            after exp is queued; return this pair's PV matmuls as new pending."""
            if ci not in out_tiles:
                out_tiles[ci] = ops.tile([128, 512], F32, tag="outc",
                                         name=f"outc{ci}")
            out_pc = out_tiles[ci][0:H + 1, :]
            nsb = 4 * ci + 4
            cl, cr = ci * 512, (ci + 1) * 512
            sbe, sbo = 2 * p, 2 * p + 1
            re, ro = sbe - 4 * ci, sbo - 4 * ci
            st = stps.tile([128, 1024], F32, tag="st")
            if re < 0:
                # fully below the diagonal: full-width S, fp8 exp, 1 DR PV
                nc.tensor.matmul(
                    st[:, 0:512],
                    ka[0:64, sbe * 128:(sbe + 1) * 128], qa[0:64, cl:cr],
                    start=True, stop=True,
                )
                nc.tensor.matmul(
                    st[:, 512:1024],
                    ka[0:64, sbo * 128:(sbo + 1) * 128], qa[0:64, cl:cr],
                    start=True, stop=True,
                )
                pt8 = ptpool.tile([128, 1024], FP8, tag="pt8")
                nc.scalar.activation(pt8[:], st[:], AF.Exp, scale=SCALE)
                for args, kw in pending:
                    nc.tensor.matmul(*args, **kw)
                return [
                    ((out_pc[:, :], v8_nat[:, sbe:sbe + 2, 0:H + 1],
                      pt8[:].rearrange("q (i t) -> q i t", i=2)),
                     dict(start=(sbe == 0), stop=False, perf_mode=DR,
                          skip_group_check=True)),
                ]
            # diagonal pair: trimmed S, bf16 exp + affine_select mask
            t0e, t0o = max(re, 0) * 128, max(ro, 0) * 128
            nc.tensor.matmul(
                st[:, t0e:512],
                ka[0:64, sbe * 128:(sbe + 1) * 128], qa[0:64, cl + t0e:cr],
                start=True, stop=True,
            )
            nc.tensor.matmul(
                st[:, 512 + t0o:1024],
                ka[0:64, sbo * 128:(sbo + 1) * 128], qa[0:64, cl + t0o:cr],
                start=True, stop=True,
            )
            pt = ptpool.tile([128, 1024], BF16, tag="pt")
            nc.scalar.activation(
                pt[:, t0e:512], st[:, t0e:512], AF.Exp, scale=SCALE)
            nc.scalar.activation(
                pt[:, 512 + t0o:1024], st[:, 512 + t0o:1024],
                AF.Exp, scale=SCALE)
            nc.gpsimd.affine_select(
                out=pt[:, t0e:t0e + 128], in_=pt[:, t0e:t0e + 128],
                compare_op=ALU.is_ge, fill=0.0,
                base=0, pattern=[[1, 128]], channel_multiplier=-1,
            )
            nc.gpsimd.affine_select(
                out=pt[:, 512 + t0o:512 + t0o + 128],
                in_=pt[:, 512 + t0o:512 + t0o + 128],
                compare_op=ALU.is_ge, fill=0.0,
                base=0, pattern=[[1, 128]], channel_multiplier=-1,
            )
            for args, kw in pending:
                nc.tensor.matmul(*args, **kw)
            return [
                ((out_pc[:, t0e:512], v_nat[:, sbe, 0:H + 1],
                  pt[:, t0e:512]),
                 dict(start=(sbe == 0), stop=False, skip_group_check=True)),
                ((out_pc[:, t0o:512], v_nat[:, sbo, 0:H + 1],
                  pt[:, 512 + t0o:1024]),
                 dict(start=False, stop=(sbo == nsb - 1),
                      skip_group_check=True)),
            ]

        def emit_attn_core(ci):
            pending = []
            for p in range(2 * ci + 2):
                pending = emit_attn_pair(ci, p, pending)
            for args, kw in pending:
                nc.tensor.matmul(*args, **kw)

        def emit_attn_multi(cis):
            # interleave pair streams so no chunk's S matmuls queue behind
            # another chunk's exp-gated PV matmuls in the PE FIFO.
            order = []
            idx = 0
            while any(idx < 2 * ci + 2 for ci in cis):
                for ci in cis:
                    if idx < 2 * ci + 2:
                        order.append((ci, idx))
                idx += 1
            pending = []
            for ci, p in order:
                pending = emit_attn_pair(ci, p, pending)
            for args, kw in pending:
                nc.tensor.matmul(*args, **kw)

        def emit_attn_out(ci):
            out_tile = out_tiles[ci]
            out_pc = out_tile[0:H + 1, :]
            o_c = opool.tile([H + 1, 512], F32, tag="osb")
            nc.vector.tensor_copy(o_c[:], out_pc[:])
            # reuse the same PSUM bank for the transposed result
            fin4 = out_tile[:, 0:4 * (H + 1)].rearrange(
                "q (r h) -> q r h", h=H + 1)
            for rr in range(4):
                nc.tensor.transpose(
                    fin4[:, rr, :],
                    o_c[:, rr * 128:(rr + 1) * 128],
                    ident[0:H + 1, 0:H + 1],
                )
            rcp = fpool.tile([128, 4, 1], F32, tag="rcp")
            nc.vector.reciprocal(rcp[:], fin4[:, :, H:H + 1])
            nc.vector.tensor_tensor(
                o_out[:, ci * 4:(ci + 1) * 4, :], fin4[:, :, 0:H],
                rcp[:].to_broadcast([128, 4, H]), op=ALU.mult,
            )
            nc.sync.dma_start(
                outR[:, ci * 4:(ci + 1) * 4, :],
                o_out[:, ci * 4:(ci + 1) * 4, :],
            )
            del out_tiles[ci]

        emit_warm(14)
        emit_qkv(0)
        emit_attn_core(0)
        emit_qkv(1)
        emit_attn_core(1)
        emit_attn_out(0)
        emit_attn_out(1)
        emit_qkv(2)
        emit_qkv(3)
        emit_attn_multi((2, 3))
        emit_attn_out(2)
        emit_attn_out(3)


BF = ml_dtypes.bfloat16
F8 = ml_dtypes.float8_e4m3


def prep_weights(Wq, Wk, Wv):
    Wq = np.asarray(Wq, dtype=np.float32)
    Wk = np.asarray(Wk, dtype=np.float32)
    Wv = np.asarray(Wv, dtype=np.float32)

    def w8(W):
        # [128, KT, 2, H] with [p, kt, i, h] = W[256kt + 2p + i, h]
        return np.ascontiguousarray(
            W.astype(F8).reshape(KT, 128, 2, H).transpose(1, 0, 2, 3))

    wv = np.ascontiguousarray(
        Wv.astype(BF).reshape(NCT, 128, H).transpose(1, 0, 2))
    return {"wq8": w8(Wq), "wk8": w8(Wk), "wv": wv}


def prep_x(xb):
    xb = np.asarray(xb, dtype=np.float32)
    # xt [128, NCH, NCT, 512] bf16: [p, ci, ct, tw] = x[512ci+tw, 128ct+p]
    xt = np.ascontiguousarray(
        xb.astype(BF).reshape(NCH, 512, NCT, 128).transpose(3, 0, 2, 1))
    # x8 [128, NCH, KT, 512, 2] fp8: [p, ci, kt, tw, i] = x[512ci+tw, 256kt+2p+i]
    x8 = np.ascontiguousarray(
        xb.astype(F8).reshape(NCH, 512, KT, 128, 2).transpose(3, 0, 2, 1, 4))
    return {"x8": x8, "xt": xt}


def make_in_maps(x, Wq, Wk, Wv):
    wmap = prep_weights(Wq, Wk, Wv)
    return [{**prep_x(x[b]), **wmap} for b in range(B)]


_NC = None


def kernel(x, Wq, Wk, Wv):
    global _NC
    if _NC is None:
        _NC = build_nc()
    in_maps = make_in_maps(x, Wq, Wk, Wv)
    res = run_bass_kernel_spmd(_NC, in_maps, core_ids=list(range(B)))
    return np.stack([res.results[b]["out"] for b in range(B)], axis=0)


# revision 8
# speedup vs baseline: 1.7328x; 1.1733x over previous
"""Single-head causal attention on 8 TRN2 NeuronCores — v25.

Problem: x[B=8, T=2048, C=1024], Wq/Wk/Wv[C, H=64] (fp32)
  q = x@Wq; k = x@Wk; v = x@Wv
  wei = softmax(mask(q k^T * C^-0.5)); out = wei @ v       -> [B, T, H]

Sharding: data-parallel over batch, one batch element per core.

v25 redesign vs v24:
  - x is marshaled HOST-side: uploaded pre-transposed (x^T) so the
    device needs no fp32 load, no DVE cast, and no xbar DMA transposes
    (which serialize against all other DMA traffic on mode switches).
    Two copies go up: bf16 x^T [128c, NCT, T] (4MB) for the v
    projection + S operands, and an fp8e4 even/odd-c byte-packed
    x^T [128, KT, T, 2] (2MB) for the q/k projections.
  - q/k projections run as fp8 DoubleRow matmuls (0.5 cyc/row): the
    byte-packed layout puts c=2p+i at (partition p, byte i), matching
    DoubleRow's [K, 2, N] two-subtile contraction exactly.
  - PV: fully-below-diagonal pairs use fp8 DoubleRow (exp -> pt8 fp8
    directly on ACT; v8 cast of v), one matmul per pair; diagonal
    pairs keep the bf16 path with affine_select masking.  Rows of
    chunk 0 stay all-bf16 (out[0]=v[0] exactly -> fp8 v would put ~6%
    error there; for t>=512 the softmax averaging buries it).
  - v_nat comes from PE transposes (bf16 identity), not the DMA xbar.
  - Weights are host-packed (fp8 DoubleRow layout / bf16) and loaded
    via gpsimd SWDGE so the scalar HWDGE ring is x-only.
Measured rel err (numpy emulation): 6.3e-3 vs 2e-2 gate.
"""
import sys

sys.path.insert(0, "/opt/trn_rl_repo")

import numpy as np
import ml_dtypes

import concourse.bass as bass
import concourse.mybir as mybir
import concourse.tile as tile
from concourse import bacc
from concourse.bass_utils import run_bass_kernel_spmd
from concourse.masks import make_identity

B, T, C, H = 8, 2048, 1024, 64
NTT = T // 128   # 16 t-tiles
NCT = C // 128   # 8  c-tiles (bf16 path)
KT = C // 256    # 4  doublerow c-tiles (fp8 path)
NCH = T // 512   # 4  t-chunks
SCALE = float(C) ** -0.5

F32 = mybir.dt.float32
BF16 = mybir.dt.bfloat16
FP8 = mybir.dt.float8e4
DR = mybir.MatmulPerfMode.DoubleRow


def build_nc(reps=1):
    nc = bacc.Bacc("TRN2", target_bir_lowering=False, debug=False,
                   dynamic_dma_scratch_size=49152)
    x8D = nc.dram_tensor("x8", [128, NCH, KT, 512, 2], FP8,
                         kind="ExternalInput").ap()
    xtD = nc.dram_tensor("xt", [128, NCH, NCT, 512], BF16,
                         kind="ExternalInput").ap()
    wq8D = nc.dram_tensor("wq8", [128, KT, 2, H], FP8, kind="ExternalInput").ap()
    wk8D = nc.dram_tensor("wk8", [128, KT, 2, H], FP8, kind="ExternalInput").ap()
    wvD = nc.dram_tensor("wv", [128, NCT, H], BF16, kind="ExternalInput").ap()
    outD = nc.dram_tensor("out", [T, H], F32, kind="ExternalOutput").ap()

    AF = mybir.ActivationFunctionType

    with tile.TileContext(nc) as tc:
        with (
            tc.tile_pool(name="const", bufs=1) as cpool,
            tc.tile_pool(name="xin", bufs=1) as xpool,
            tc.tile_pool(name="qk", bufs=1) as qkpool,
            tc.tile_pool(name="pt", bufs=4) as ptpool,
            tc.tile_pool(name="osb", bufs=3) as opool,
            tc.tile_pool(name="fin", bufs=2) as fpool,
        ):
            # x loads lead the scalar HWDGE ring: per-chunk pieces so the
            # first QKV can start ~4us into the load stream.
            # All input DMA rides the sync HWDGE ring (descriptor
            # generation on SP keeps the Activation engine free for exp).
            # W first (tiny), then all fp8 x8 chunks (q/k projections can
            # front-run), then the bf16 xt chunks.  Chunk-contiguous per
            # partition for 4KB/8KB bursts.
            wq8 = cpool.tile([128, KT, 2, H], FP8)
            wk8 = cpool.tile([128, KT, 2, H], FP8)
            wv = cpool.tile([128, NCT, H], BF16)
            nc.sync.dma_start(wq8[:], wq8D)
            nc.sync.dma_start(wk8[:], wk8D)
            nc.sync.dma_start(wv[:], wvD)
            x8 = xpool.tile([128, NCH, KT, 512, 2], FP8, tag="x8")
            xt = xpool.tile([128, NCH, NCT, 512], BF16, tag="xt")
            for ci in range(NCH):
                nc.sync.dma_start(x8[:, ci], x8D[:, ci])
            for ci in range(NCH):
                nc.sync.dma_start(xt[:, ci], xtD[:, ci])


            ident = cpool.tile([128, 128], F32)
            make_identity(nc, ident[:])
            identb = cpool.tile([128, 64], BF16)
            nc.vector.tensor_copy(identb[64:128, :], ident[64:128, 64:128])

            scrap = cpool.tile([128, 1], F32)
            # first Exp triggers ACT_TABLE_LOAD early (after load issue so
            # it doesn't head-block the x loads in any DMA path)
            nc.scalar.activation(scrap[:], ident[:, 0:1], AF.Exp)

            for rep in range(reps):
                emit_body(nc, tc, outD,
                          (x8, xt, wq8, wk8, wv, ident, identb),
                          (qkpool, ptpool, opool, fpool))

    nc.compile()
    return nc


def emit_body(nc, tc, outD, consts, pools):
    AF = mybir.ActivationFunctionType
    ALU = mybir.AluOpType
    x8, xt, wq8, wk8, wv, ident, identb = consts
    qkpool, ptpool, opool, fpool = pools

    qa = qkpool.tile([64, T], BF16, tag="qa")
    ka = qkpool.tile([64, T], BF16, tag="ka")
    va = qkpool.tile([128, T], BF16, tag="va")    # rows 64:128 hold v^T
    v_nat = qkpool.tile([128, NTT, H + 1], BF16, tag="vnat")
    VP8 = 80   # fp8 v stride: dual-fp8 LDWEIGHTS needs even, 16-aligned subtile step
    v8_nat = qkpool.tile([128, NTT, VP8], FP8, tag="v8nat")
    nc.gpsimd.memset(v_nat[:, :, H:H + 1], 1.0)
    nc.gpsimd.memset(v8_nat[:, :, H:H + 1], 1.0)
    o_out = fpool.tile([128, NTT, H], F32, tag="oout")
    outR = outD.rearrange("(g p) h -> p g h", p=128)

    with (
        tc.tile_pool(name="qkvps", bufs=1, space="PSUM") as qkvps,
        tc.tile_pool(name="vtps", bufs=1, space="PSUM") as vtps,
        tc.tile_pool(name="stps", bufs=2, space="PSUM") as stps,
        tc.tile_pool(name="ops", bufs=1, space="PSUM") as ops,
    ):
        def emit_warm(n):
            # fp32 identity matmuls keep the PE p-state ramping from ~7us
            # until the first QKV; gated only on make_identity.
            warm = stps.tile([128, 1024], F32, tag="st")
            for _ in range(n):
                nc.tensor.matmul(
                    warm[0:128, 0:128], ident[:], ident[:],
                    start=True, stop=True,
                )

        def emit_qk(ci):
            # q, k: fp8 DoubleRow over 4 256-wide c-tiles
            sl = slice(ci * 512, (ci + 1) * 512)
            ps_q = qkvps.tile([64, 512], F32, tag="psq")
            for kt in range(KT):
                nc.tensor.matmul(
                    ps_q[:], wq8[:, kt, :, :],
                    x8[:, ci, kt, :, :].rearrange("p t i -> p i t"),
                    start=(kt == 0), stop=(kt == KT - 1), perf_mode=DR,
                )
            nc.vector.tensor_copy(qa[:, sl], ps_q[:])
            ps_k = qkvps.tile([64, 512], F32, tag="psv")
            for kt in range(KT):
                nc.tensor.matmul(
                    ps_k[:], wk8[:, kt, :, :],
                    x8[:, ci, kt, :, :].rearrange("p t i -> p i t"),
                    start=(kt == 0), stop=(kt == KT - 1), perf_mode=DR,
                )
            nc.vector.tensor_copy(ka[:, sl], ps_k[:])

        def emit_v(ci):
            sl = slice(ci * 512, (ci + 1) * 512)
            ps_v = qkvps.tile([64, 512], F32, tag="psv")
            for c in range(NCT):
                nc.tensor.matmul(
                    ps_v[:], wv[:, c, :], xt[:, ci, c, :],
                    start=(c == 0), stop=(c == NCT - 1),
                )
            nc.vector.tensor_copy(va[64:128, sl], ps_v[:])
            # v^T -> v natural via PE transposes (no DMA xbar!)
            ps_vt = vtps.tile([128, 4, H], BF16, tag="vt")
            for r in range(4):
                nc.tensor.transpose(
                    ps_vt[:, r, :],
                    va[64:128, ci * 512 + r * 128: ci * 512 + (r + 1) * 128],
                    identb[64:128, :],
                )
            nc.vector.tensor_copy(v_nat[:, 4 * ci:4 * ci + 4, 0:H], ps_vt[:])
            nc.vector.tensor_copy(v8_nat[:, 4 * ci:4 * ci + 4, 0:H], ps_vt[:])

        out_tiles = {}

        def emit_attn_pair(ci, p):
            """Emit S+exp for pair p of chunk ci; return PV matmul ops (the
            caller threads them into the PE stream at explicit points)."""
            if ci not in out_tiles:
                out_tiles[ci] = ops.tile([128, 512], F32, tag="outc",
                                         name=f"outc{ci}")
            out_pc = out_tiles[ci][0:H + 1, :]
            nsb = 4 * ci + 4
            cl, cr = ci * 512, (ci + 1) * 512
            sbe, sbo = 2 * p, 2 * p + 1
            re, ro = sbe - 4 * ci, sbo - 4 * ci
            st = stps.tile([128, 1024], F32, tag="st")
            if re < 0:
                # fully below the diagonal: full-width S, fp8 exp, 1 DR PV
                nc.tensor.matmul(
                    st[:, 0:512],
                    ka[0:64, sbe * 128:(sbe + 1) * 128], qa[0:64, cl:cr],
                    start=True, stop=True,
                )
                nc.tensor.matmul(
                    st[:, 512:1024],
                    ka[0:64, sbo * 128:(sbo + 1) * 128], qa[0:64, cl:cr],
                    start=True, stop=True,
                )
                pt8 = ptpool.tile([128, 1024], FP8, tag="pt8")
                nc.scalar.activation(pt8[:], st[:], AF.Exp, scale=SCALE)
                return [
                    ((out_pc[:, :], v8_nat[:, sbe:sbe + 2, 0:H + 1],
                      pt8[:].rearrange("q (i t) -> q i t", i=2)),
                     dict(start=(sbe == 0), stop=False, perf_mode=DR,
                          skip_group_check=True)),
                ]
            # diagonal pair: trimmed S, bf16 exp + affine_select mask
            t0e, t0o = max(re, 0) * 128, max(ro, 0) * 128
            nc.tensor.matmul(
                st[:, t0e:512],
                ka[0:64, sbe * 128:(sbe + 1) * 128], qa[0:64, cl + t0e:cr],
                start=True, stop=True,
            )
            nc.tensor.matmul(
                st[:, 512 + t0o:1024],
                ka[0:64, sbo * 128:(sbo + 1) * 128], qa[0:64, cl + t0o:cr],
                start=True, stop=True,
            )
            pt = ptpool.tile([128, 1024], BF16, tag="pt")
            nc.scalar.activation(
                pt[:, t0e:512], st[:, t0e:512], AF.Exp, scale=SCALE)
            nc.scalar.activation(
                pt[:, 512 + t0o:1024], st[:, 512 + t0o:1024],
                AF.Exp, scale=SCALE)
            nc.gpsimd.affine_select(
                out=pt[:, t0e:t0e + 128], in_=pt[:, t0e:t0e + 128],
                compare_op=ALU.is_ge, fill=0.0,
                base=0, pattern=[[1, 128]], channel_multiplier=-1,
            )
            nc.gpsimd.affine_select(
                out=pt[:, 512 + t0o:512 + t0o + 128],
                in_=pt[:, 512 + t0o:512 + t0o + 128],
                compare_op=ALU.is_ge, fill=0.0,
                base=0, pattern=[[1, 128]], channel_multiplier=-1,
            )
            return [
                ((out_pc[:, t0e:512], v_nat[:, sbe, 0:H + 1],
                  pt[:, t0e:512]),
                 dict(start=(sbe == 0), stop=False, skip_group_check=True)),
                ((out_pc[:, t0o:512], v_nat[:, sbo, 0:H + 1],
                  pt[:, 512 + t0o:1024]),
                 dict(start=False, stop=(sbo == nsb - 1),
                      skip_group_check=True)),
            ]

        def emit_attn_out(ci):
            out_tile = out_tiles[ci]
            out_pc = out_tile[0:H + 1, :]
            o_c = opool.tile([H + 1, 512], F32, tag="osb")
            nc.vector.tensor_copy(o_c[:], out_pc[:])
            # reuse the same PSUM bank for the transposed result
            fin4 = out_tile[:, 0:4 * (H + 1)].rearrange(
                "q (r h) -> q r h", h=H + 1)
            for rr in range(4):
                nc.tensor.transpose(
                    fin4[:, rr, :],
                    o_c[:, rr * 128:(rr + 1) * 128],
                    ident[0:H + 1, 0:H + 1],
                )
            rcp = fpool.tile([128, 4, 1], F32, tag="rcp")
            nc.vector.reciprocal(rcp[:], fin4[:, :, H:H + 1])
            nc.vector.tensor_tensor(
                o_out[:, ci * 4:(ci + 1) * 4, :], fin4[:, :, 0:H],
                rcp[:].to_broadcast([128, 4, H]), op=ALU.mult,
            )
            nc.sync.dma_start(
                outR[:, ci * 4:(ci + 1) * 4, :],
                o_out[:, ci * 4:(ci + 1) * 4, :],
            )
            del out_tiles[ci]

        # Global schedule: q/k projections front-run on the fp8 stream
        # (x8 loads land first), then one pair-stream in chunk order at
        # exp cadence (st pool depth 2 throttles S two pairs ahead of
        # ACT), with v-projections / PV groups / finalizes threaded in at
        # their data-ready points so neither PE nor ACT head-blocks.
        pvq = {ci: [] for ci in range(NCH)}

        def P(ci, p):
            pvq[ci] += emit_attn_pair(ci, p)

        def flush_pv(ci):
            for args, kw in pvq[ci]:
                nc.tensor.matmul(*args, **kw)
            pvq[ci] = []

        emit_warm(5)
        for ci in range(NCH):
            emit_qk(ci)
        P(0, 0); P(0, 1)
        P(1, 0); P(1, 1)
        emit_v(0)
        P(1, 2); P(1, 3)
        flush_pv(0); emit_attn_out(0)
        P(2, 0); P(2, 1)
        emit_v(1)
        P(2, 2); P(2, 3)
        flush_pv(1); emit_attn_out(1)
        P(2, 4); P(2, 5)
        emit_v(2)
        P(3, 0); P(3, 1)
        flush_pv(2); emit_attn_out(2)
        P(3, 2); P(3, 3)
        emit_v(3)
        P(3, 4); P(3, 5)
        P(3, 6); P(3, 7)
        flush_pv(3); emit_attn_out(3)


BF = ml_dtypes.bfloat16
F8 = ml_dtypes.float8_e4m3


def prep_weights(Wq, Wk, Wv):
    Wq = np.asarray(Wq, dtype=np.float32)
    Wk = np.asarray(Wk, dtype=np.float32)
    Wv = np.asarray(Wv, dtype=np.float32)

    def w8(W):
        # [128, KT, 2, H] with [p, kt, i, h] = W[256kt + 2p + i, h]
        return np.ascontiguousarray(
            W.astype(F8).reshape(KT, 128, 2, H).transpose(1, 0, 2, 3))

    wv = np.ascontiguousarray(
        Wv.astype(BF).reshape(NCT, 128, H).transpose(1, 0, 2))
    return {"wq8": w8(Wq), "wk8": w8(Wk), "wv": wv}


def prep_x(xb):
    xb = np.asarray(xb, dtype=np.float32)
    # xt [128, NCH, NCT, 512] bf16: [p, ci, ct, tw] = x[512ci+tw, 128ct+p]
    xt = np.ascontiguousarray(
        xb.astype(BF).reshape(NCH, 512, NCT, 128).transpose(3, 0, 2, 1))
    # x8 [128, NCH, KT, 512, 2] fp8: [p, ci, kt, tw, i] = x[512ci+tw, 256kt+2p+i]
    x8 = np.ascontiguousarray(
        xb.astype(F8).reshape(NCH, 512, KT, 128, 2).transpose(3, 0, 2, 1, 4))
    return {"x8": x8, "xt": xt}


def make_in_maps(x, Wq, Wk, Wv):
    wmap = prep_weights(Wq, Wk, Wv)
    return [{**prep_x(x[b]), **wmap} for b in range(B)]


_NC = None


def kernel(x, Wq, Wk, Wv):
    global _NC
    if _NC is None:
        _NC = build_nc()
    in_maps = make_in_maps(x, Wq, Wk, Wv)
    res = run_bass_kernel_spmd(_NC, in_maps, core_ids=list(range(B)))
    return np.stack([res.results[b]["out"] for b in range(B)], axis=0)


# revision 9
# speedup vs baseline: 1.7600x; 1.0157x over previous
"""Single-head causal attention on 8 TRN2 NeuronCores — v25.

Problem: x[B=8, T=2048, C=1024], Wq/Wk/Wv[C, H=64] (fp32)
  q = x@Wq; k = x@Wk; v = x@Wv
  wei = softmax(mask(q k^T * C^-0.5)); out = wei @ v       -> [B, T, H]

Sharding: data-parallel over batch, one batch element per core.

v25 redesign vs v24:
  - x is marshaled HOST-side: uploaded pre-transposed (x^T) so the
    device needs no fp32 load, no DVE cast, and no xbar DMA transposes
    (which serialize against all other DMA traffic on mode switches).
    Two copies go up: bf16 x^T [128c, NCT, T] (4MB) for the v
    projection + S operands, and an fp8e4 even/odd-c byte-packed
    x^T [128, KT, T, 2] (2MB) for the q/k projections.
  - q/k projections run as fp8 DoubleRow matmuls (0.5 cyc/row): the
    byte-packed layout puts c=2p+i at (partition p, byte i), matching
    DoubleRow's [K, 2, N] two-subtile contraction exactly.
  - PV: fully-below-diagonal pairs use fp8 DoubleRow (exp -> pt8 fp8
    directly on ACT; v8 cast of v), one matmul per pair; diagonal
    pairs keep the bf16 path with affine_select masking.  Rows of
    chunk 0 stay all-bf16 (out[0]=v[0] exactly -> fp8 v would put ~6%
    error there; for t>=512 the softmax averaging buries it).
  - v_nat comes from PE transposes (bf16 identity), not the DMA xbar.
  - Weights are host-packed (fp8 DoubleRow layout / bf16) and loaded
    via gpsimd SWDGE so the scalar HWDGE ring is x-only.
Measured rel err (numpy emulation): 6.3e-3 vs 2e-2 gate.
"""
import sys

sys.path.insert(0, "/opt/trn_rl_repo")

import numpy as np
import ml_dtypes

import concourse.bass as bass
import concourse.mybir as mybir
import concourse.tile as tile
from concourse import bacc
from concourse.bass_utils import run_bass_kernel_spmd
from concourse.masks import make_identity

B, T, C, H = 8, 2048, 1024, 64
NTT = T // 128   # 16 t-tiles
NCT = C // 128   # 8  c-tiles (bf16 path)
KT = C // 256    # 4  doublerow c-tiles (fp8 path)
NCH = T // 512   # 4  t-chunks
SCALE = float(C) ** -0.5

F32 = mybir.dt.float32
BF16 = mybir.dt.bfloat16
FP8 = mybir.dt.float8e4
DR = mybir.MatmulPerfMode.DoubleRow


def build_nc(reps=1):
    nc = bacc.Bacc("TRN2", target_bir_lowering=False, debug=False,
                   dynamic_dma_scratch_size=49152)
    x8D = nc.dram_tensor("x8", [128, NCH, KT, 512, 2], FP8,
                         kind="ExternalInput").ap()
    xtD = nc.dram_tensor("xt", [128, NCH, NCT, 512], BF16,
                         kind="ExternalInput").ap()
    wqk8D = nc.dram_tensor("wqk8", [128, 2, KT, 2, H], FP8,
                           kind="ExternalInput").ap()
    wvD = nc.dram_tensor("wv", [128, NCT, H], BF16, kind="ExternalInput").ap()
    outD = nc.dram_tensor("out", [T, H], F32, kind="ExternalOutput").ap()

    AF = mybir.ActivationFunctionType

    with tile.TileContext(nc) as tc:
        with (
            tc.tile_pool(name="const", bufs=1) as cpool,
            tc.tile_pool(name="xin", bufs=1) as xpool,
            tc.tile_pool(name="qk", bufs=1) as qkpool,
            tc.tile_pool(name="pt", bufs=4) as ptpool,
            tc.tile_pool(name="osb", bufs=3) as opool,
            tc.tile_pool(name="fin", bufs=2) as fpool,
        ):
            # x loads lead the scalar HWDGE ring: per-chunk pieces so the
            # first QKV can start ~4us into the load stream.
            # All input DMA rides the sync HWDGE ring (descriptor
            # generation on SP keeps the Activation engine free for exp).
            # W first (tiny), then all fp8 x8 chunks (q/k projections can
            # front-run), then the bf16 xt chunks.  Chunk-contiguous per
            # partition for 4KB/8KB bursts.
            wqk8 = cpool.tile([128, 2, KT, 2, H], FP8)
            wv = cpool.tile([128, NCT, H], BF16)
            x8 = xpool.tile([128, NCH, KT, 512, 2], FP8, tag="x8")
            xt = xpool.tile([128, NCH, NCT, 512], BF16, tag="xt")
            nc.sync.dma_start(wqk8[:], wqk8D)
            for ci in range(NCH):
                nc.sync.dma_start(x8[:, ci], x8D[:, ci])
            nc.sync.dma_start(wv[:], wvD)
            for ci in range(NCH):
                nc.sync.dma_start(xt[:, ci], xtD[:, ci])


            ident = cpool.tile([128, 128], F32)
            make_identity(nc, ident[:])
            identb = cpool.tile([128, 64], BF16)
            nc.vector.tensor_copy(identb[64:128, :], ident[64:128, 64:128])

            scrap = cpool.tile([128, 1], F32)
            # first Exp triggers ACT_TABLE_LOAD early (after load issue so
            # it doesn't head-block the x loads in any DMA path)
            nc.scalar.activation(scrap[:], ident[:, 0:1], AF.Exp)

            for rep in range(reps):
                emit_body(nc, tc, outD,
                          (x8, xt, wqk8, wv, ident, identb),
                          (qkpool, ptpool, opool, fpool))

    nc.compile()
    return nc


def emit_body(nc, tc, outD, consts, pools):
    AF = mybir.ActivationFunctionType
    ALU = mybir.AluOpType
    x8, xt, wqk8, wv, ident, identb = consts
    qkpool, ptpool, opool, fpool = pools

    qa = qkpool.tile([64, T], BF16, tag="qa")
    ka = qkpool.tile([64, T], BF16, tag="ka")
    va = qkpool.tile([128, T], BF16, tag="va")    # rows 64:128 hold v^T
    v_nat = qkpool.tile([128, NTT, H + 1], BF16, tag="vnat")
    VP8 = 80   # fp8 v stride: dual-fp8 LDWEIGHTS needs even, 16-aligned subtile step
    v8_nat = qkpool.tile([128, NTT, VP8], FP8, tag="v8nat")
    nc.gpsimd.memset(v_nat[:, :, H:H + 1], 1.0)
    nc.gpsimd.memset(v8_nat[:, :, H:H + 1], 1.0)
    o_out = fpool.tile([128, NTT, H], F32, tag="oout")
    outR = outD.rearrange("(g p) h -> p g h", p=128)

    with (
        tc.tile_pool(name="qkvps", bufs=1, space="PSUM") as qkvps,
        tc.tile_pool(name="vtps", bufs=1, space="PSUM") as vtps,
        tc.tile_pool(name="stps", bufs=2, space="PSUM") as stps,
        tc.tile_pool(name="ops", bufs=1, space="PSUM") as ops,
    ):
        def emit_warm(n):
            # fp32 identity matmuls keep the PE p-state ramping from ~7us
            # until the first QKV; gated only on make_identity.
            warm = stps.tile([128, 1024], F32, tag="st")
            for _ in range(n):
                nc.tensor.matmul(
                    warm[0:128, 0:128], ident[:], ident[:],
                    start=True, stop=True,
                )

        def emit_qk(ci):
            # q, k: fp8 DoubleRow over 4 256-wide c-tiles
            sl = slice(ci * 512, (ci + 1) * 512)
            ps_q = qkvps.tile([64, 512], F32, tag="psq")
            for kt in range(KT):
                nc.tensor.matmul(
                    ps_q[:], wqk8[:, 0, kt, :, :],
                    x8[:, ci, kt, :, :].rearrange("p t i -> p i t"),
                    start=(kt == 0), stop=(kt == KT - 1), perf_mode=DR,
                )
            nc.vector.tensor_copy(qa[:, sl], ps_q[:])
            ps_k = qkvps.tile([64, 512], F32, tag="psv")
            for kt in range(KT):
                nc.tensor.matmul(
                    ps_k[:], wqk8[:, 1, kt, :, :],
                    x8[:, ci, kt, :, :].rearrange("p t i -> p i t"),
                    start=(kt == 0), stop=(kt == KT - 1), perf_mode=DR,
                )
            nc.vector.tensor_copy(ka[:, sl], ps_k[:])

        def emit_v(ci):
            sl = slice(ci * 512, (ci + 1) * 512)
            ps_v = qkvps.tile([64, 512], F32, tag="psv")
            for c in range(NCT):
                nc.tensor.matmul(
                    ps_v[:], wv[:, c, :], xt[:, ci, c, :],
                    start=(c == 0), stop=(c == NCT - 1),
                )
            nc.vector.tensor_copy(va[64:128, sl], ps_v[:])
            # v^T -> v natural via PE transposes (no DMA xbar!)
            ps_vt = vtps.tile([128, 4, H], BF16, tag="vt")
            for r in range(4):
                nc.tensor.transpose(
                    ps_vt[:, r, :],
                    va[64:128, ci * 512 + r * 128: ci * 512 + (r + 1) * 128],
                    identb[64:128, :],
                )
            nc.vector.tensor_copy(v_nat[:, 4 * ci:4 * ci + 4, 0:H], ps_vt[:])
            nc.vector.tensor_copy(v8_nat[:, 4 * ci:4 * ci + 4, 0:H], ps_vt[:])

        out_tiles = {}

        def emit_attn_pair(ci, p):
            """Emit S+exp for pair p of chunk ci; return PV matmul ops (the
            caller threads them into the PE stream at explicit points)."""
            if ci not in out_tiles:
                out_tiles[ci] = ops.tile([128, 512], F32, tag="outc",
                                         name=f"outc{ci}")
            out_pc = out_tiles[ci][0:H + 1, :]
            nsb = 4 * ci + 4
            cl, cr = ci * 512, (ci + 1) * 512
            sbe, sbo = 2 * p, 2 * p + 1
            re, ro = sbe - 4 * ci, sbo - 4 * ci
            st = stps.tile([128, 1024], F32, tag="st")
            if re < 0:
                # fully below the diagonal: full-width S, fp8 exp, 1 DR PV
                nc.tensor.matmul(
                    st[:, 0:512],
                    ka[0:64, sbe * 128:(sbe + 1) * 128], qa[0:64, cl:cr],
                    start=True, stop=True,
                )
                nc.tensor.matmul(
                    st[:, 512:1024],
                    ka[0:64, sbo * 128:(sbo + 1) * 128], qa[0:64, cl:cr],
                    start=True, stop=True,
                )
                pt8 = ptpool.tile([128, 1024], FP8, tag="pt8")
                nc.scalar.activation(pt8[:], st[:], AF.Exp, scale=SCALE)
                return [
                    ((out_pc[:, :], v8_nat[:, sbe:sbe + 2, 0:H + 1],
                      pt8[:].rearrange("q (i t) -> q i t", i=2)),
                     dict(start=(sbe == 0), stop=False, perf_mode=DR,
                          skip_group_check=True)),
                ]
            # diagonal pair: trimmed S, bf16 exp + affine_select mask
            t0e, t0o = max(re, 0) * 128, max(ro, 0) * 128
            nc.tensor.matmul(
                st[:, t0e:512],
                ka[0:64, sbe * 128:(sbe + 1) * 128], qa[0:64, cl + t0e:cr],
                start=True, stop=True,
            )
            nc.tensor.matmul(
                st[:, 512 + t0o:1024],
                ka[0:64, sbo * 128:(sbo + 1) * 128], qa[0:64, cl + t0o:cr],
                start=True, stop=True,
            )
            pt = ptpool.tile([128, 1024], BF16, tag="pt")
            nc.scalar.activation(
                pt[:, t0e:512], st[:, t0e:512], AF.Exp, scale=SCALE)
            nc.scalar.activation(
                pt[:, 512 + t0o:1024], st[:, 512 + t0o:1024],
                AF.Exp, scale=SCALE)
            nc.gpsimd.affine_select(
                out=pt[:, t0e:t0e + 128], in_=pt[:, t0e:t0e + 128],
                compare_op=ALU.is_ge, fill=0.0,
                base=0, pattern=[[1, 128]], channel_multiplier=-1,
            )
            nc.gpsimd.affine_select(
                out=pt[:, 512 + t0o:512 + t0o + 128],
                in_=pt[:, 512 + t0o:512 + t0o + 128],
                compare_op=ALU.is_ge, fill=0.0,
                base=0, pattern=[[1, 128]], channel_multiplier=-1,
            )
            return [
                ((out_pc[:, t0e:512], v_nat[:, sbe, 0:H + 1],
                  pt[:, t0e:512]),
                 dict(start=(sbe == 0), stop=False, skip_group_check=True)),
                ((out_pc[:, t0o:512], v_nat[:, sbo, 0:H + 1],
                  pt[:, 512 + t0o:1024]),
                 dict(start=False, stop=(sbo == nsb - 1),
                      skip_group_check=True)),
            ]

        def emit_attn_out(ci):
            out_tile = out_tiles[ci]
            out_pc = out_tile[0:H + 1, :]
            o_c = opool.tile([H + 1, 512], F32, tag="osb")
            nc.vector.tensor_copy(o_c[:], out_pc[:])
            # reuse the same PSUM bank for the transposed result
            fin4 = out_tile[:, 0:4 * (H + 1)].rearrange(
                "q (r h) -> q r h", h=H + 1)
            for rr in range(4):
                nc.tensor.transpose(
                    fin4[:, rr, :],
                    o_c[:, rr * 128:(rr + 1) * 128],
                    ident[0:H + 1, 0:H + 1],
                )
            rcp = fpool.tile([128, 4, 1], F32, tag="rcp")
            nc.vector.reciprocal(rcp[:], fin4[:, :, H:H + 1])
            nc.vector.tensor_tensor(
                o_out[:, ci * 4:(ci + 1) * 4, :], fin4[:, :, 0:H],
                rcp[:].to_broadcast([128, 4, H]), op=ALU.mult,
            )
            nc.sync.dma_start(
                outR[:, ci * 4:(ci + 1) * 4, :],
                o_out[:, ci * 4:(ci + 1) * 4, :],
            )
            del out_tiles[ci]

        # Global schedule: q/k projections front-run on the fp8 stream
        # (x8 loads land first), then one pair-stream in chunk order at
        # exp cadence (st pool depth 2 throttles S two pairs ahead of
        # ACT), with v-projections / PV groups / finalizes threaded in at
        # their data-ready points so neither PE nor ACT head-blocks.
        pvq = {ci: [] for ci in range(NCH)}

        def P(ci, p):
            pvq[ci] += emit_attn_pair(ci, p)

        def flush_pv(ci):
            for args, kw in pvq[ci]:
                nc.tensor.matmul(*args, **kw)
            pvq[ci] = []

        emit_warm(5)
        emit_qk(0)
        emit_qk(1)
        P(0, 0); P(0, 1)
        emit_qk(2)
        P(1, 0); P(1, 1)
        emit_qk(3)
        P(1, 2); P(1, 3)
        emit_v(0)
        P(2, 0); P(2, 1)
        flush_pv(0); emit_attn_out(0)
        P(2, 2); P(2, 3)
        emit_v(1)
        P(2, 4); P(2, 5)
        flush_pv(1); emit_attn_out(1)
        P(3, 0); P(3, 1)
        emit_v(2)
        P(3, 2); P(3, 3)
        flush_pv(2); emit_attn_out(2)
        P(3, 4); P(3, 5)
        emit_v(3)
        P(3, 6); P(3, 7)
        flush_pv(3); emit_attn_out(3)


BF = ml_dtypes.bfloat16
F8 = ml_dtypes.float8_e4m3


def prep_weights(Wq, Wk, Wv):
    Wq = np.asarray(Wq, dtype=np.float32)
    Wk = np.asarray(Wk, dtype=np.float32)
    Wv = np.asarray(Wv, dtype=np.float32)

    def w8(W):
        # [128, KT, 2, H] with [p, kt, i, h] = W[256kt + 2p + i, h]
        return np.ascontiguousarray(
            W.astype(F8).reshape(KT, 128, 2, H).transpose(1, 0, 2, 3))

    wv = np.ascontiguousarray(
        Wv.astype(BF).reshape(NCT, 128, H).transpose(1, 0, 2))
    wqk8 = np.ascontiguousarray(np.stack([w8(Wq), w8(Wk)], axis=1))
    return {"wqk8": wqk8, "wv": wv}


def prep_x(xb):
    xb = np.asarray(xb, dtype=np.float32)
    # xt [128, NCH, NCT, 512] bf16: [p, ci, ct, tw] = x[512ci+tw, 128ct+p]
    xt = np.ascontiguousarray(
        xb.astype(BF).reshape(NCH, 512, NCT, 128).transpose(3, 0, 2, 1))
    # x8 [128, NCH, KT, 512, 2] fp8: [p, ci, kt, tw, i] = x[512ci+tw, 256kt+2p+i]
    x8 = np.ascontiguousarray(
        xb.astype(F8).reshape(NCH, 512, KT, 128, 2).transpose(3, 0, 2, 1, 4))
    return {"x8": x8, "xt": xt}


def make_in_maps(x, Wq, Wk, Wv):
    wmap = prep_weights(Wq, Wk, Wv)
    return [{**prep_x(x[b]), **wmap} for b in range(B)]


_NC = None


def kernel(x, Wq, Wk, Wv):
    global _NC
    if _NC is None:
        _NC = build_nc()
    in_maps = make_in_maps(x, Wq, Wk, Wv)
    res = run_bass_kernel_spmd(_NC, in_maps, core_ids=list(range(B)))
    return np.stack([res.results[b]["out"] for b in range(B)], axis=0)
